# revision 19
# baseline (speedup 1.0000x reference)
"""Fused QKV+RoPE+GQA-attention kernel for Trainium2, sharded over 8 NeuronCores.

Sharding: data-parallel over batch (B=2), tensor-parallel over heads (4 groups of
8 q-heads / 2 kv-heads per batch element).  Each core computes its output slice
out[b, :, g*1024:(g+1)*1024] independently; no collectives.

Per-core pipeline (n=2048 seq, c=4096 model, d=128 head dim), all matmul
operands in bf16 (full PE rate at any moving width, half the DMA/SBUF of
fp32r):
  - Weights live SBUF-resident for the whole kernel (12.6 MB bf16, DMA'd once
    up front) instead of being re-streamed per position chunk.
  - QKV^T projection:  Q^T/K^T = W X^T (feature-major, head_dim on partitions),
    V = X W^T (position-major).
  - RoPE without partition shifts: rot_half(q) = sign * (R q) where R is the
    swap-halves permutation applied via one extra matmul per head-chunk, and the
    sign is folded into the sin table on host.
  - Attention: S^T = K_rope Q_rope^T (k on partitions, q on free dim), causal
    structure exploited at 128-tile granularity (per-q-subtile AV depth), the
    real attention-mask applied only inside diagonal 128x128 blocks, softmax
    without max-subtraction (logits are O(10); exp(-3.4e38) = 0 exactly on the
    ACT LUT), denominator from a ones column augmented into V inside the same
    AV matmul (129-wide moving, only the relevant kv head's V), normalization
    fused into the PSUM drain as a per-partition scale (times observation mask).
  - Emission interleaves attention(chunk i-1) with projection(chunk i): the PE
    executes its queue in order, so alternating independent streams fills each
    stream's dependency bubbles and keeps the HAM clock gate warm.
"""

from contextlib import ExitStack

import ml_dtypes
import numpy as np

import concourse.tile as tile
from concourse import bacc, mybir
from concourse.bass_utils import run_bass_kernel_spmd

F32 = mybir.dt.float32
BF16 = mybir.dt.bfloat16

B, N, C = 2, 2048, 4096
NUM_HEADS, KV_HEADS, HEAD_DIM = 32, 8, 128
GROUPS = 4                     # head groups per batch element
QH = NUM_HEADS // GROUPS       # 8 q heads per core
KVH = KV_HEADS // GROUPS       # 2 kv heads per core
N_CORES = B * GROUPS

NCHUNK = 512                   # seq positions per pass
NCHUNKS = N // NCHUNK          # 4
CC = C // 128                  # 32 contraction chunks
NT = N // 128                  # 16 position tiles
VW = 130                       # v block width in vaug: [v(128) | one | pad]


def _build_program():
    nc = bacc.Bacc("TRN2", target_bir_lowering=False, debug=False, num_devices=N_CORES)

    xt = nc.dram_tensor("xt", [NCHUNKS, 8, 128, 4, NCHUNK], BF16, kind="ExternalInput").ap()
    wt = nc.dram_tensor("wt", [6, 128, 8, 4, 256], BF16, kind="ExternalInput").ap()
    cosT = nc.dram_tensor("cosT", [128, N], F32, kind="ExternalInput").ap()
    sinmT = nc.dram_tensor("sinmT", [128, N], F32, kind="ExternalInput").ap()
    maskd = nc.dram_tensor("maskd", [NCHUNKS, 128, 4, 128], F32, kind="ExternalInput").ap()
    obs = nc.dram_tensor("obs", [128, NT], F32, kind="ExternalInput").ap()
    rmat = nc.dram_tensor("rmat", [128, 128], BF16, kind="ExternalInput").ap()
    out = nc.dram_tensor("out", [N, QH * 128], BF16, kind="ExternalOutput").ap()

    with tile.TileContext(nc) as tc, ExitStack() as ctx:
        singles = ctx.enter_context(tc.tile_pool(name="singles", bufs=1))
        xpool = ctx.enter_context(tc.tile_pool(name="xt", bufs=9))
        rpool = ctx.enter_context(tc.tile_pool(name="rope", bufs=2))
        cspool = ctx.enter_context(tc.tile_pool(name="cs", bufs=2))
        qtpool = ctx.enter_context(tc.tile_pool(name="qt", bufs=2))
        ptpool = ctx.enter_context(tc.tile_pool(name="pt", bufs=4))
        mpool = ctx.enter_context(tc.tile_pool(name="mask", bufs=1))
        opool = ctx.enter_context(tc.tile_pool(name="outp", bufs=4))
        spool = ctx.enter_context(tc.tile_pool(name="small", bufs=8))
        pp_proj = ctx.enter_context(tc.tile_pool(name="pp_proj", bufs=2, space="PSUM"))
        pp_misc = ctx.enter_context(tc.tile_pool(name="pp_misc", bufs=2, space="PSUM"))
        pp_av = ctx.enter_context(tc.tile_pool(name="pp_av", bufs=4, space="PSUM"))

        # ---- persistent tiles (DMAs emitted after chunk 0 xt kick off) ----
        rmat_sb = singles.tile([128, 128], BF16, tag="rmat")
        obs_sb = singles.tile([128, NT], F32, tag="obs")
        ones_sb = singles.tile([128, NT, 2], BF16, tag="ones")
        kt_res = singles.tile([128, KVH, N], BF16, tag="ktres")      # K^T rope'd
        vaug = singles.tile([128, NT, 2 * VW], BF16, tag="vaug")     # V + ones cols
        wsb = [singles.tile([128, 8, 4, 256], BF16, tag=f"wt{fg}", name=f"wsb{fg}")
               for fg in range(6)]

        def emit_singles_early():
            # quarter-granularity fg0 DMAs so the first matmuls start early
            for q in range(4):
                nc.sync.dma_start(out=wsb[0][:, 2 * q:2 * q + 2],
                                  in_=wt[0][:, 2 * q:2 * q + 2])
            nc.sync.dma_start(out=rmat_sb, in_=rmat)

        def emit_singles_late():
            # remaining weights split across the sync and gpsimd queues
            for fg in (1, 2, 3):
                nc.sync.dma_start(out=wsb[fg], in_=wt[fg])
            for fg in (5, 4):
                nc.gpsimd.dma_start(out=wsb[fg], in_=wt[fg])
            nc.gpsimd.dma_start(out=obs_sb, in_=obs)
            nc.vector.memset(ones_sb, 1.0)
            nc.scalar.copy(vaug[:, :, 128:130], ones_sb)
            nc.scalar.copy(vaug[:, :, VW + 128:VW + 130], ones_sb)

        def rope_pair(ps_a, ps_b, dest_a, dest_b, cos_c, sinm_c):
            """dest = ps*cos + rot_half(ps)*sinm for two head chunks.

            rot_half is a partition rotation by 64 done with two SBUF->SBUF
            DMA copies (sign folded into sinm on host) — no PE involvement."""
            for ps, dest, tg in ((ps_a, dest_a, "a"), (ps_b, dest_b, "b")):
                qq = rpool.tile([128, NCHUNK], BF16, tag="qq" + tg)
                nc.vector.tensor_scalar_mul(qq, ps, 1.0)
                rot = rpool.tile([128, NCHUNK], BF16, tag="rot" + tg)
                nc.sync.dma_start(out=rot[0:64, :], in_=qq[64:128, :])
                nc.sync.dma_start(out=rot[64:128, :], in_=qq[0:64, :])
                tcs = rpool.tile([128, NCHUNK], F32, tag="tcs" + tg, bufs=1)
                nc.vector.tensor_mul(tcs, ps, cos_c)
                # qq is dead after the rotate; reuse its slot for the sin product
                nc.vector.tensor_mul(qq, rot, sinm_c)
                nc.vector.tensor_add(dest, tcs, qq)

        def chunk_dmas(qc):
            """Emit the input DMAs for chunk qc; returns chunk state."""
            n0 = qc * NCHUNK
            engs = ([nc.gpsimd, nc.scalar, nc.sync] if qc == 0
                    else [nc.gpsimd, nc.scalar])
            xsub = []
            for j in range(8):
                t = xpool.tile([128, 4, NCHUNK], BF16, tag="xt", name=f"xt{qc}_{j}")
                # spread the tile fetches across parallel DMA queues
                engs[j % len(engs)].dma_start(out=t, in_=xt[qc, j])
                xsub.append(t)
            cos_c = cspool.tile([128, NCHUNK], F32, tag="cos", name=f"cos{qc}")
            nc.gpsimd.dma_start(out=cos_c, in_=cosT[:, n0:n0 + NCHUNK])
            sinm_c = cspool.tile([128, NCHUNK], F32, tag="sinm", name=f"sinm{qc}")
            nc.gpsimd.dma_start(out=sinm_c, in_=sinmT[:, n0:n0 + NCHUNK])
            qt_chunk = qtpool.tile([128, QH, NCHUNK], BF16, tag="qt", name=f"qt{qc}")
            return dict(n0=n0, xsub=xsub, cos=cos_c, sinm=sinm_c, qt=qt_chunk)

        def proj_units(qc, st):
            """7 emission units: 5 feature-group c-loops (+deferred rope), 2 V passes."""
            n0, xsub = st["n0"], st["xsub"]

            def xc(c):
                return xsub[c // 4][:, c % 4, :]

            pend = {}

            def emit_fg(fg, half):
                if half == 0:
                    pend[fg] = (
                        pp_proj.tile([128, NCHUNK], F32, tag="proj", name=f"pa{qc}_{fg}"),
                        pp_proj.tile([128, NCHUNK], F32, tag="proj", name=f"pb{qc}_{fg}"))
                ps_a, ps_b = pend[fg]
                for cq in range(4 * half, 4 * half + 4):
                    for i in range(4):
                        c = 4 * cq + i
                        nc.tensor.matmul(ps_a, wsb[fg][:, cq, i, 0:128], xc(c),
                                         start=(c == 0), stop=(c == CC - 1))
                        nc.tensor.matmul(ps_b, wsb[fg][:, cq, i, 128:256], xc(c),
                                         start=(c == 0), stop=(c == CC - 1))

            def emit_rope(fg):
                ps_a, ps_b = pend.pop(fg)
                if fg < 4:
                    d_a = st["qt"][:, 2 * fg, :]
                    d_b = st["qt"][:, 2 * fg + 1, :]
                else:
                    d_a = kt_res[:, 0, n0:n0 + NCHUNK]
                    d_b = kt_res[:, 1, n0:n0 + NCHUNK]
                rope_pair(ps_a, ps_b, d_a, d_b, st["cos"], st["sinm"])

            vps = {}

            def emit_vhalf(p, half):
                if half == 0:
                    vps[p] = [pp_proj.tile([128, 256], F32, tag="proj",
                                           name=f"pv{qc}_{p}_{i}") for i in range(2)]
                ps = vps[p]
                for cq in range(4 * half, 4 * half + 4):
                    for i in range(4):
                        c = 4 * cq + i
                        for k in range(2):
                            ns = 2 * p + k
                            nc.tensor.matmul(ps[k], xc(c)[:, 128 * ns:128 * ns + 128],
                                             wsb[5][:, cq, i, :],
                                             start=(c == 0), stop=(c == CC - 1))
                if half == 1:
                    for k in range(2):
                        ns = 2 * p + k
                        nt_i = 4 * qc + ns
                        dst = vaug[:, nt_i, :].rearrange("p (h w) -> p h w", h=2)[:, :, 0:128]
                        nc.scalar.copy(dst, ps[k].rearrange("p (h w) -> p h w", h=2))

            units = [lambda: emit_fg(0, 0), lambda: emit_fg(0, 1)]
            for fg in range(1, 5):
                units.append(lambda fg=fg: (emit_rope(fg - 1), emit_fg(fg, 0)))
                units.append(lambda fg=fg: emit_fg(fg, 1))
            units.append(lambda: (emit_rope(4), emit_vhalf(0, 0)))
            units.append(lambda: emit_vhalf(0, 1))
            units.append(lambda: emit_vhalf(1, 0))
            units.append(lambda: emit_vhalf(1, 1))
            return units

        def attn_units(qc, st):
            """9 emission units: mask/out setup + 8 heads; last head DMAs out."""
            n0, qt_chunk = st["n0"], st["qt"]
            nk = 4 * qc + 4
            shared = {}

            def emit_pre():
                m_sb = mpool.tile([128, 4, 128], F32, tag="mask", name=f"m{qc}")
                nc.gpsimd.dma_start(out=m_sb, in_=maskd[qc])
                shared["m"] = m_sb
                shared["outs"] = [opool.tile([128, QH * 128], BF16, tag="out",
                                             name=f"out_s{qc}_{s}") for s in range(4)]

            def emit_head(h):
                kv = h // (QH // KVH)
                vbase = kv * VW
                m_sb, out_s = shared["m"], shared["outs"]
                pt_tiles = {}
                # last chunk has no projection interleaved: borrow the idle
                # proj psum pool too, so 4 S tiles can be in flight
                last = qc == NCHUNKS - 1
                STAG = 3 if last else 2

                def qk_step(kt):
                    # columns left of q-sub kd are never consumed (per-s AV depth)
                    kd = kt - 4 * qc
                    lo = 128 * kd if kd > 0 else 0
                    spsum = pp_proj if (last and kt % 2) else pp_misc
                    ps_s = spsum.tile([128, NCHUNK], F32, tag="misc" if spsum is pp_misc
                                      else "proj", name=f"s{qc}_{h}_{kt}")
                    nc.tensor.matmul(ps_s[:, lo:], kt_res[:, kv, 128 * kt:128 * kt + 128],
                                     qt_chunk[:, h, lo:], start=True, stop=True)
                    if kd >= 0:
                        # causal mask inside the diagonal 128x128 block only
                        nc.vector.tensor_add(ps_s[:, 128 * kd:128 * kd + 128],
                                             ps_s[:, 128 * kd:128 * kd + 128],
                                             m_sb[:, kd, :])
                    pt = ptpool.tile([128, NCHUNK], BF16, tag="pt", name=f"pt{qc}_{h}_{kt}")
                    nc.scalar.activation(out=pt[:, lo:], in_=ps_s[:, lo:],
                                         func=mybir.ActivationFunctionType.Exp)
                    pt_tiles[kt] = pt

                # one accumulation group per PSUM bank: start_tensor_calc
                # resets the whole bank, so groups cannot share one
                ps_o = [pp_av.tile([128, VW - 1], F32, tag="av", name=f"o{qc}_{h}_{s}")
                        for s in range(4)]

                def av_step(kt):
                    pt = pt_tiles.pop(kt)
                    for s in range(4):
                        if kt > 4 * qc + s:
                            continue  # beyond this q-sub's causal depth
                        nc.tensor.matmul(ps_o[s],
                                         pt[:, 128 * s:128 * s + 128],
                                         vaug[:, kt, vbase:vbase + VW - 1],
                                         start=(kt == 4 * qc),
                                         stop=(kt == (4 * qc - 1 if qc else s)))

                # diagonal tiles first: their DVE mask hop pipelines while the
                # stagger is filling, leaving the steady state mask-free
                kts = list(range(4 * qc, nk)) + list(range(4 * qc))
                for idx in range(nk + STAG):
                    if idx < nk:
                        qk_step(kts[idx])
                    if idx >= STAG:
                        av_step(kts[idx - STAG])

                # drain the AV banks with one DVE copy each so the next head's
                # accumulation can start; normalize from SBUF off the PE and
                # ACT paths (gpsimd cannot read PSUM)
                avsb = opool.tile([128, 4, VW - 1], F32, tag="avsb",
                                  name=f"avsb{qc}_{h}", bufs=2)
                for s in range(4):
                    nc.vector.tensor_scalar_mul(avsb[:, s, :], ps_o[s], 1.0)
                for s in range(4):
                    den = spool.tile([128, 1], F32, tag="den")
                    nc.vector.reciprocal(den, avsb[:, s, 128:129])
                    sc = spool.tile([128, 1], F32, tag="sc")
                    nc.vector.tensor_mul(sc, den, obs_sb[:, 4 * qc + s:4 * qc + s + 1])
                    nc.vector.tensor_scalar_mul(out_s[s][:, 128 * h:128 * h + 128],
                                                avsb[:, s, 0:128], sc)
                if h == QH // 2 - 1:
                    # first half of the heads is done: drain those columns early
                    for s in range(4):
                        nc.scalar.dma_start(
                            out=out[n0 + 128 * s:n0 + 128 * (s + 1), 0:QH * 64],
                            in_=out_s[s][:, 0:QH * 64])
                if h == QH - 1:
                    for s in range(4):
                        nc.scalar.dma_start(
                            out=out[n0 + 128 * s:n0 + 128 * (s + 1), QH * 64:],
                            in_=out_s[s][:, QH * 64:])

            units = [emit_pre]
            for h in range(QH):
                units.append(lambda h=h: emit_head(h))
            return units

        # ---- pipelined emission: attention(qc-1) interleaved with proj(qc) ----
        emit_singles_early()
        states = {0: chunk_dmas(0)}
        emit_singles_late()
        for u in proj_units(0, states[0]):
            u()
        for qc in range(1, NCHUNKS + 1):
            au = attn_units(qc - 1, states[qc - 1])
            pu = []
            if qc < NCHUNKS:
                states[qc] = chunk_dmas(qc)
                pu = proj_units(qc, states[qc])
            # proportional merge of the two unit streams
            tagged = [((i + 0.5) / len(au), 0, u) for i, u in enumerate(au)]
            tagged += [((j + 0.5) / len(pu), 1, u) for j, u in enumerate(pu)]
            for _, _, u in sorted(tagged, key=lambda t: (t[0], t[1])):
                u()
            del states[qc - 1]

    nc.compile()
    return nc


_NC_CACHE = None


def _get_program():
    global _NC_CACHE
    if _NC_CACHE is None:
        _NC_CACHE = _build_program()
    return _NC_CACHE


def _prep_core_inputs(input, weight, cos_cached, sin_cached, attention_mask,
                      position_ids, observation_mask):
    """Build the 8 per-core input maps (host-side shard + layout + bf16 cast)."""
    input = np.asarray(input, dtype=np.float32)
    weight = np.asarray(weight, dtype=np.float32)
    cos_cached = np.asarray(cos_cached, dtype=np.float32)
    sin_cached = np.asarray(sin_cached, dtype=np.float32)
    attention_mask = np.asarray(attention_mask, dtype=np.float32)
    position_ids = np.asarray(position_ids)
    observation_mask = np.asarray(observation_mask)

    scale = 1.0 / np.sqrt(HEAD_DIM)
    rmat = np.zeros((128, 128), dtype=np.float32)
    idx = np.arange(128)
    rmat[idx, (idx + 64) % 128] = 1.0
    rmat = rmat.astype(ml_dtypes.bfloat16)

    in_maps = []
    for core in range(N_CORES):
        b, g = core // GROUPS, core % GROUPS
        xtT = input[b].T.astype(ml_dtypes.bfloat16)                    # [C, N]
        xt = np.ascontiguousarray(
            xtT.reshape(8, 4, 128, NCHUNKS, NCHUNK).transpose(3, 0, 2, 1, 4))

        wq = weight[g * QH * 128:(g + 1) * QH * 128] * scale           # [1024, C]
        k_off = NUM_HEADS * 128
        wk = weight[k_off + g * KVH * 128:k_off + (g + 1) * KVH * 128]  # [256, C]
        v_off = k_off + KV_HEADS * 128
        wv = weight[v_off + g * KVH * 128:v_off + (g + 1) * KVH * 128]  # [256, C]
        wtT = np.concatenate([wq, wk, wv], axis=0).T.astype(ml_dtypes.bfloat16)
        wt = np.ascontiguousarray(
            wtT.reshape(8, 4, 128, 6, 256).transpose(3, 2, 0, 1, 4))   # [6,128,8,4,256]

        pos = position_ids[b]
        cosT = np.ascontiguousarray(cos_cached[0, 0][pos].T)           # [128, N]
        sinmT = np.ascontiguousarray(sin_cached[0, 0][pos].T)
        sinmT[:64] = -sinmT[:64]

        m = attention_mask[b, 0]                                       # [N, N]
        maskd = np.stack([
            np.stack([m[t * 128:(t + 1) * 128, t * 128:(t + 1) * 128].T
                      for t in range(4 * qc, 4 * qc + 4)]).transpose(1, 0, 2)
            for qc in range(NCHUNKS)])                                 # [4, 128, 4, 128]
        maskd = np.ascontiguousarray(maskd)

        obsf = np.ascontiguousarray(
            (observation_mask[b] == 1).astype(np.float32).reshape(NT, 128).T)

        in_maps.append(dict(xt=xt, wt=wt, cosT=cosT, sinmT=sinmT, maskd=maskd,
                            obs=obsf, rmat=rmat))
    return in_maps


def run(inputs: dict, trace: bool = False):
    """Run the sharded kernel; returns (full_output [B*N, C] fp32, BassKernelResults)."""
    nc = _get_program()
    in_maps = _prep_core_inputs(**inputs)
    res = run_bass_kernel_spmd(nc, in_maps, core_ids=list(range(N_CORES)), trace=trace)
    full = np.empty((B, N, C), dtype=np.float32)
    for core in range(N_CORES):
        b, g = core // GROUPS, core % GROUPS
        full[b, :, g * QH * 128:(g + 1) * QH * 128] = res.results[core]["out"].astype(
            np.float32)
    return full.reshape(B * N, C), res


def kernel(**inputs) -> np.ndarray:
    out, _ = run(inputs)
    return out


# revision 21
# speedup vs baseline: 1.1562x; 1.1562x over previous
"""Fused QKV+RoPE+GQA-attention kernel for Trainium2, sharded over 8 NeuronCores.

Sharding: data-parallel over batch (B=2), tensor-parallel over heads (4 groups of
8 q-heads / 2 kv-heads per batch element).  Each core computes its output slice
out[b, :, g*1024:(g+1)*1024] independently; no collectives.

Per-core pipeline (n=2048 seq, c=4096 model, d=128 head dim), all matmul
operands in bf16 (full PE rate at any moving width, half the DMA/SBUF of
fp32r):
  - Weights live SBUF-resident for the whole kernel (12.6 MB bf16, DMA'd once
    up front) instead of being re-streamed per position chunk.
  - QKV^T projection:  Q^T/K^T = W X^T (feature-major, head_dim on partitions),
    V = X W^T (position-major).
  - RoPE without partition shifts: rot_half(q) = sign * (R q) where R is the
    swap-halves permutation applied via one extra matmul per head-chunk, and the
    sign is folded into the sin table on host.
  - Attention: S^T = K_rope Q_rope^T (k on partitions, q on free dim), causal
    structure exploited at 128-tile granularity (per-q-subtile AV depth), the
    real attention-mask applied only inside diagonal 128x128 blocks, softmax
    without max-subtraction (logits are O(10); exp(-3.4e38) = 0 exactly on the
    ACT LUT), denominator from a ones column augmented into V inside the same
    AV matmul (129-wide moving, only the relevant kv head's V), normalization
    fused into the PSUM drain as a per-partition scale (times observation mask).
  - Emission interleaves attention(chunk i-1) with projection(chunk i): the PE
    executes its queue in order, so alternating independent streams fills each
    stream's dependency bubbles and keeps the HAM clock gate warm.
"""

from contextlib import ExitStack

import ml_dtypes
import numpy as np

import concourse.tile as tile
from concourse import bacc, mybir
from concourse.bass_utils import run_bass_kernel_spmd

F32 = mybir.dt.float32
BF16 = mybir.dt.bfloat16

B, N, C = 2, 2048, 4096
NUM_HEADS, KV_HEADS, HEAD_DIM = 32, 8, 128
GROUPS = 4                     # head groups per batch element
QH = NUM_HEADS // GROUPS       # 8 q heads per core
KVH = KV_HEADS // GROUPS       # 2 kv heads per core
N_CORES = B * GROUPS

NCHUNK = 512                   # seq positions per pass
NCHUNKS = N // NCHUNK          # 4
CC = C // 128                  # 32 contraction chunks
NT = N // 128                  # 16 position tiles
VW = 130                       # v block width in vaug: [v(128) | one | pad]


def _build_program():
    nc = bacc.Bacc("TRN2", target_bir_lowering=False, debug=False, num_devices=N_CORES)

    xt = nc.dram_tensor("xt", [NCHUNKS, 8, 128, 4, NCHUNK], BF16, kind="ExternalInput").ap()
    wt = nc.dram_tensor("wt", [6, 128, 8, 4, 256], BF16, kind="ExternalInput").ap()
    cosT = nc.dram_tensor("cosT", [128, N], F32, kind="ExternalInput").ap()
    sinmT = nc.dram_tensor("sinmT", [128, N], F32, kind="ExternalInput").ap()
    maskd = nc.dram_tensor("maskd", [NCHUNKS, 128, 4, 128], F32, kind="ExternalInput").ap()
    obs = nc.dram_tensor("obs", [128, NT], F32, kind="ExternalInput").ap()
    rmat = nc.dram_tensor("rmat", [128, 128], BF16, kind="ExternalInput").ap()
    out = nc.dram_tensor("out", [N, QH * 128], BF16, kind="ExternalOutput").ap()

    with tile.TileContext(nc) as tc, ExitStack() as ctx:
        singles = ctx.enter_context(tc.tile_pool(name="singles", bufs=1))
        xpool = ctx.enter_context(tc.tile_pool(name="xt", bufs=9))
        rpool = ctx.enter_context(tc.tile_pool(name="rope", bufs=2))
        cspool = ctx.enter_context(tc.tile_pool(name="cs", bufs=2))
        qtpool = ctx.enter_context(tc.tile_pool(name="qt", bufs=2))
        ptpool = ctx.enter_context(tc.tile_pool(name="pt", bufs=4))
        mpool = ctx.enter_context(tc.tile_pool(name="mask", bufs=1))
        opool = ctx.enter_context(tc.tile_pool(name="outp", bufs=4))
        spool = ctx.enter_context(tc.tile_pool(name="small", bufs=8))
        pp_proj = ctx.enter_context(tc.tile_pool(name="pp_proj", bufs=2, space="PSUM"))
        pp_misc = ctx.enter_context(tc.tile_pool(name="pp_misc", bufs=2, space="PSUM"))
        pp_av = ctx.enter_context(tc.tile_pool(name="pp_av", bufs=4, space="PSUM"))

        # ---- persistent tiles (DMAs emitted after chunk 0 xt kick off) ----
        rmat_sb = singles.tile([128, 128], BF16, tag="rmat")
        obs_sb = singles.tile([128, NT], F32, tag="obs")
        ones_sb = singles.tile([128, NT, 2], BF16, tag="ones")
        kt_res = singles.tile([128, KVH, N], BF16, tag="ktres")      # K^T rope'd
        vaug = singles.tile([128, NT, 2 * VW], BF16, tag="vaug")     # V + ones cols
        wsb = [singles.tile([128, 8, 4, 256], BF16, tag=f"wt{fg}", name=f"wsb{fg}")
               for fg in range(6)]

        def emit_singles_early():
            # quarter-granularity fg0 DMAs so the first matmuls start early
            for q in range(4):
                nc.sync.dma_start(out=wsb[0][:, 2 * q:2 * q + 2],
                                  in_=wt[0][:, 2 * q:2 * q + 2])
            nc.sync.dma_start(out=rmat_sb, in_=rmat)

        def emit_singles_late():
            # fg1 now; fg2/fg3 deferred into the chunk-0 unit stream so the
            # sync queue never backlogs ahead of later small transfers
            nc.sync.dma_start(out=wsb[1], in_=wt[1])
            for fg in (5, 4):
                nc.gpsimd.dma_start(out=wsb[fg], in_=wt[fg])
            nc.gpsimd.dma_start(out=obs_sb, in_=obs)
            nc.vector.memset(ones_sb, 1.0)
            nc.scalar.copy(vaug[:, :, 128:130], ones_sb)
            nc.scalar.copy(vaug[:, :, VW + 128:VW + 130], ones_sb)

        def rope_pair(ps_a, ps_b, dest_a, dest_b, cos_c, sinm_c):
            """dest = ps*cos + (R @ ps)*sinm for two head chunks."""
            qq_a = rpool.tile([128, NCHUNK], BF16, tag="qq")
            nc.vector.tensor_scalar_mul(qq_a, ps_a, 1.0)
            qq_b = rpool.tile([128, NCHUNK], BF16, tag="qq2")
            nc.vector.tensor_scalar_mul(qq_b, ps_b, 1.0)
            pr_a = pp_misc.tile([128, NCHUNK], F32, tag="misc")
            nc.tensor.matmul(pr_a, rmat_sb, qq_a, start=True, stop=True)
            pr_b = pp_misc.tile([128, NCHUNK], F32, tag="misc")
            nc.tensor.matmul(pr_b, rmat_sb, qq_b, start=True, stop=True)
            for ps, pr, qq, dest, tg in ((ps_a, pr_a, qq_a, dest_a, "a"),
                                         (ps_b, pr_b, qq_b, dest_b, "b")):
                tcs = rpool.tile([128, NCHUNK], F32, tag="tcs" + tg, bufs=1)
                nc.vector.tensor_mul(tcs, ps, cos_c)
                # qq is dead after the R-matmul; reuse its slot for the sin product
                nc.vector.tensor_mul(qq, pr, sinm_c)
                nc.vector.tensor_add(dest, tcs, qq)

        def chunk_dmas(qc):
            """Emit the input DMAs for chunk qc; returns chunk state."""
            n0 = qc * NCHUNK
            engs = ([nc.gpsimd, nc.scalar, nc.sync] if qc == 0
                    else [nc.gpsimd, nc.scalar])
            xsub = []
            for j in range(8):
                t = xpool.tile([128, 4, NCHUNK], BF16, tag="xt", name=f"xt{qc}_{j}")
                # spread the tile fetches across parallel DMA queues
                engs[j % len(engs)].dma_start(out=t, in_=xt[qc, j])
                xsub.append(t)
            cos_c = cspool.tile([128, NCHUNK], F32, tag="cos", name=f"cos{qc}")
            nc.gpsimd.dma_start(out=cos_c, in_=cosT[:, n0:n0 + NCHUNK])
            sinm_c = cspool.tile([128, NCHUNK], F32, tag="sinm", name=f"sinm{qc}")
            nc.gpsimd.dma_start(out=sinm_c, in_=sinmT[:, n0:n0 + NCHUNK])
            qt_chunk = qtpool.tile([128, QH, NCHUNK], BF16, tag="qt", name=f"qt{qc}")
            return dict(n0=n0, xsub=xsub, cos=cos_c, sinm=sinm_c, qt=qt_chunk)

        def proj_units(qc, st):
            """7 emission units: 5 feature-group c-loops (+deferred rope), 2 V passes."""
            n0, xsub = st["n0"], st["xsub"]

            def xc(c):
                return xsub[c // 4][:, c % 4, :]

            pend = {}

            def emit_fg(fg, half):
                if half == 0:
                    pend[fg] = (
                        pp_proj.tile([128, NCHUNK], F32, tag="proj", name=f"pa{qc}_{fg}"),
                        pp_proj.tile([128, NCHUNK], F32, tag="proj", name=f"pb{qc}_{fg}"))
                ps_a, ps_b = pend[fg]
                for cq in range(4 * half, 4 * half + 4):
                    for i in range(4):
                        c = 4 * cq + i
                        nc.tensor.matmul(ps_a, wsb[fg][:, cq, i, 0:128], xc(c),
                                         start=(c == 0), stop=(c == CC - 1))
                        nc.tensor.matmul(ps_b, wsb[fg][:, cq, i, 128:256], xc(c),
                                         start=(c == 0), stop=(c == CC - 1))

            def emit_rope(fg):
                ps_a, ps_b = pend.pop(fg)
                if fg < 4:
                    d_a = st["qt"][:, 2 * fg, :]
                    d_b = st["qt"][:, 2 * fg + 1, :]
                else:
                    d_a = kt_res[:, 0, n0:n0 + NCHUNK]
                    d_b = kt_res[:, 1, n0:n0 + NCHUNK]
                rope_pair(ps_a, ps_b, d_a, d_b, st["cos"], st["sinm"])

            vps = {}

            def emit_vhalf(p, half):
                if half == 0:
                    vps[p] = [pp_proj.tile([128, 256], F32, tag="proj",
                                           name=f"pv{qc}_{p}_{i}") for i in range(2)]
                ps = vps[p]
                for cq in range(4 * half, 4 * half + 4):
                    for i in range(4):
                        c = 4 * cq + i
                        for k in range(2):
                            ns = 2 * p + k
                            nc.tensor.matmul(ps[k], xc(c)[:, 128 * ns:128 * ns + 128],
                                             wsb[5][:, cq, i, :],
                                             start=(c == 0), stop=(c == CC - 1))
                if half == 1:
                    for k in range(2):
                        ns = 2 * p + k
                        nt_i = 4 * qc + ns
                        dst = vaug[:, nt_i, :].rearrange("p (h w) -> p h w", h=2)[:, :, 0:128]
                        nc.scalar.copy(dst, ps[k].rearrange("p (h w) -> p h w", h=2))

            units = [lambda: emit_fg(0, 0), lambda: emit_fg(0, 1)]
            for fg in range(1, 5):
                units.append(lambda fg=fg: (emit_rope(fg - 1), emit_fg(fg, 0)))
                units.append(lambda fg=fg: emit_fg(fg, 1))
            units.append(lambda: (emit_rope(4), emit_vhalf(0, 0)))
            units.append(lambda: emit_vhalf(0, 1))
            units.append(lambda: emit_vhalf(1, 0))
            units.append(lambda: emit_vhalf(1, 1))
            if qc == 0:
                # deferred weight fetches: emitted two units before first use
                for idx, fg in ((2, 2), (4, 3)):
                    prev = units[idx]
                    units[idx] = (lambda prev=prev, fg=fg:
                                  (nc.sync.dma_start(out=wsb[fg], in_=wt[fg]), prev()))
            return units

        def attn_units(qc, st):
            """9 emission units: mask/out setup + 8 heads; last head DMAs out."""
            n0, qt_chunk = st["n0"], st["qt"]
            nk = 4 * qc + 4
            shared = {}

            def emit_pre():
                m_sb = mpool.tile([128, 4, 128], F32, tag="mask", name=f"m{qc}")
                nc.gpsimd.dma_start(out=m_sb, in_=maskd[qc])
                shared["m"] = m_sb
                shared["outs"] = [opool.tile([128, QH * 128], BF16, tag="out",
                                             name=f"out_s{qc}_{s}") for s in range(4)]

            def emit_head(h):
                kv = h // (QH // KVH)
                vbase = kv * VW
                m_sb, out_s = shared["m"], shared["outs"]
                pt_tiles = {}
                # last chunk has no projection interleaved: borrow the idle
                # proj psum pool too, so 4 S tiles can be in flight
                last = qc == NCHUNKS - 1
                STAG = 3 if last else 2

                def qk_step(kt):
                    # columns left of q-sub kd are never consumed (per-s AV depth)
                    kd = kt - 4 * qc
                    lo = 128 * kd if kd > 0 else 0
                    spsum = pp_proj if (last and kt % 2) else pp_misc
                    ps_s = spsum.tile([128, NCHUNK], F32, tag="misc" if spsum is pp_misc
                                      else "proj", name=f"s{qc}_{h}_{kt}")
                    nc.tensor.matmul(ps_s[:, lo:], kt_res[:, kv, 128 * kt:128 * kt + 128],
                                     qt_chunk[:, h, lo:], start=True, stop=True)
                    if kd >= 0:
                        # causal mask inside the diagonal 128x128 block only
                        nc.vector.tensor_add(ps_s[:, 128 * kd:128 * kd + 128],
                                             ps_s[:, 128 * kd:128 * kd + 128],
                                             m_sb[:, kd, :])
                    pt = ptpool.tile([128, NCHUNK], BF16, tag="pt", name=f"pt{qc}_{h}_{kt}")
                    nc.scalar.activation(out=pt[:, lo:], in_=ps_s[:, lo:],
                                         func=mybir.ActivationFunctionType.Exp)
                    pt_tiles[kt] = pt

                # one accumulation group per PSUM bank: start_tensor_calc
                # resets the whole bank, so groups cannot share one
                ps_o = [pp_av.tile([128, VW - 1], F32, tag="av", name=f"o{qc}_{h}_{s}")
                        for s in range(4)]

                def av_step(kt):
                    pt = pt_tiles.pop(kt)
                    for s in range(4):
                        if kt > 4 * qc + s:
                            continue  # beyond this q-sub's causal depth
                        nc.tensor.matmul(ps_o[s],
                                         pt[:, 128 * s:128 * s + 128],
                                         vaug[:, kt, vbase:vbase + VW - 1],
                                         start=(kt == 4 * qc),
                                         stop=(kt == (4 * qc - 1 if qc else s)))

                # diagonal tiles first: their DVE mask hop pipelines while the
                # stagger is filling, leaving the steady state mask-free
                kts = list(range(4 * qc, nk)) + list(range(4 * qc))
                for idx in range(nk + STAG):
                    if idx < nk:
                        qk_step(kts[idx])
                    if idx >= STAG:
                        av_step(kts[idx - STAG])

                # drain the AV banks with one DVE copy each so the next head's
                # accumulation can start; normalize from SBUF off the PE and
                # ACT paths (gpsimd cannot read PSUM)
                avsb = opool.tile([128, 4, VW - 1], F32, tag="avsb",
                                  name=f"avsb{qc}_{h}", bufs=2)
                for s in range(4):
                    nc.vector.tensor_scalar_mul(avsb[:, s, :], ps_o[s], 1.0)
                for s in range(4):
                    den = spool.tile([128, 1], F32, tag="den")
                    nc.vector.reciprocal(den, avsb[:, s, 128:129])
                    sc = spool.tile([128, 1], F32, tag="sc")
                    nc.vector.tensor_mul(sc, den, obs_sb[:, 4 * qc + s:4 * qc + s + 1])
                    nc.vector.tensor_scalar_mul(out_s[s][:, 128 * h:128 * h + 128],
                                                avsb[:, s, 0:128], sc)
                if h == QH // 2 - 1:
                    # first half of the heads is done: drain those columns early
                    for s in range(4):
                        nc.scalar.dma_start(
                            out=out[n0 + 128 * s:n0 + 128 * (s + 1), 0:QH * 64],
                            in_=out_s[s][:, 0:QH * 64])
                if h == QH - 1:
                    for s in range(4):
                        nc.scalar.dma_start(
                            out=out[n0 + 128 * s:n0 + 128 * (s + 1), QH * 64:],
                            in_=out_s[s][:, QH * 64:])

            units = [emit_pre]
            for h in range(QH):
                units.append(lambda h=h: emit_head(h))
            return units

        # ---- pipelined emission: attention(qc-1) interleaved with proj(qc) ----
        emit_singles_early()
        states = {0: chunk_dmas(0)}
        emit_singles_late()
        for u in proj_units(0, states[0]):
            u()
        for qc in range(1, NCHUNKS + 1):
            au = attn_units(qc - 1, states[qc - 1])
            pu = []
            if qc < NCHUNKS:
                states[qc] = chunk_dmas(qc)
                pu = proj_units(qc, states[qc])
            # proportional merge of the two unit streams
            tagged = [((i + 0.5) / len(au), 0, u) for i, u in enumerate(au)]
            tagged += [((j + 0.5) / len(pu), 1, u) for j, u in enumerate(pu)]
            for _, _, u in sorted(tagged, key=lambda t: (t[0], t[1])):
                u()
            del states[qc - 1]

    nc.compile()
    return nc


_NC_CACHE = None


def _get_program():
    global _NC_CACHE
    if _NC_CACHE is None:
        _NC_CACHE = _build_program()
    return _NC_CACHE


def _prep_core_inputs(input, weight, cos_cached, sin_cached, attention_mask,
                      position_ids, observation_mask):
    """Build the 8 per-core input maps (host-side shard + layout + bf16 cast)."""
    input = np.asarray(input, dtype=np.float32)
    weight = np.asarray(weight, dtype=np.float32)
    cos_cached = np.asarray(cos_cached, dtype=np.float32)
    sin_cached = np.asarray(sin_cached, dtype=np.float32)
    attention_mask = np.asarray(attention_mask, dtype=np.float32)
    position_ids = np.asarray(position_ids)
    observation_mask = np.asarray(observation_mask)

    scale = 1.0 / np.sqrt(HEAD_DIM)
    rmat = np.zeros((128, 128), dtype=np.float32)
    idx = np.arange(128)
    rmat[idx, (idx + 64) % 128] = 1.0
    rmat = rmat.astype(ml_dtypes.bfloat16)

    in_maps = []
    for core in range(N_CORES):
        b, g = core // GROUPS, core % GROUPS
        xtT = input[b].T.astype(ml_dtypes.bfloat16)                    # [C, N]
        xt = np.ascontiguousarray(
            xtT.reshape(8, 4, 128, NCHUNKS, NCHUNK).transpose(3, 0, 2, 1, 4))

        wq = weight[g * QH * 128:(g + 1) * QH * 128] * scale           # [1024, C]
        k_off = NUM_HEADS * 128
        wk = weight[k_off + g * KVH * 128:k_off + (g + 1) * KVH * 128]  # [256, C]
        v_off = k_off + KV_HEADS * 128
        wv = weight[v_off + g * KVH * 128:v_off + (g + 1) * KVH * 128]  # [256, C]
        wtT = np.concatenate([wq, wk, wv], axis=0).T.astype(ml_dtypes.bfloat16)
        wt = np.ascontiguousarray(
            wtT.reshape(8, 4, 128, 6, 256).transpose(3, 2, 0, 1, 4))   # [6,128,8,4,256]

        pos = position_ids[b]
        cosT = np.ascontiguousarray(cos_cached[0, 0][pos].T)           # [128, N]
        sinmT = np.ascontiguousarray(sin_cached[0, 0][pos].T)
        sinmT[:64] = -sinmT[:64]

        m = attention_mask[b, 0]                                       # [N, N]
        maskd = np.stack([
            np.stack([m[t * 128:(t + 1) * 128, t * 128:(t + 1) * 128].T
                      for t in range(4 * qc, 4 * qc + 4)]).transpose(1, 0, 2)
            for qc in range(NCHUNKS)])                                 # [4, 128, 4, 128]
        maskd = np.ascontiguousarray(maskd)

        obsf = np.ascontiguousarray(
            (observation_mask[b] == 1).astype(np.float32).reshape(NT, 128).T)

        in_maps.append(dict(xt=xt, wt=wt, cosT=cosT, sinmT=sinmT, maskd=maskd,
                            obs=obsf, rmat=rmat))
    return in_maps


def run(inputs: dict, trace: bool = False):
    """Run the sharded kernel; returns (full_output [B*N, C] fp32, BassKernelResults)."""
    nc = _get_program()
    in_maps = _prep_core_inputs(**inputs)
    res = run_bass_kernel_spmd(nc, in_maps, core_ids=list(range(N_CORES)), trace=trace)
    full = np.empty((B, N, C), dtype=np.float32)
    for core in range(N_CORES):
        b, g = core // GROUPS, core % GROUPS
        full[b, :, g * QH * 128:(g + 1) * QH * 128] = res.results[core]["out"].astype(
            np.float32)
    return full.reshape(B * N, C), res


def kernel(**inputs) -> np.ndarray:
    out, _ = run(inputs)
    return out


# revision 24
# speedup vs baseline: 1.1570x; 1.0006x over previous
"""Fused QKV+RoPE+GQA-attention kernel for Trainium2, sharded over 8 NeuronCores.

Sharding: data-parallel over batch (B=2), tensor-parallel over heads (4 groups of
8 q-heads / 2 kv-heads per batch element).  Each core computes its output slice
out[b, :, g*1024:(g+1)*1024] independently; no collectives.

Per-core pipeline (n=2048 seq, c=4096 model, d=128 head dim), all matmul
operands in bf16 (full PE rate at any moving width, half the DMA/SBUF of
fp32r):
  - Weights live SBUF-resident for the whole kernel (12.6 MB bf16, DMA'd once
    up front) instead of being re-streamed per position chunk.
  - QKV^T projection:  Q^T/K^T = W X^T (feature-major, head_dim on partitions),
    V = X W^T (position-major).
  - RoPE without partition shifts: rot_half(q) = sign * (R q) where R is the
    swap-halves permutation applied via one extra matmul per head-chunk, and the
    sign is folded into the sin table on host.
  - Attention: S^T = K_rope Q_rope^T (k on partitions, q on free dim), causal
    structure exploited at 128-tile granularity (per-q-subtile AV depth), the
    real attention-mask applied only inside diagonal 128x128 blocks, softmax
    without max-subtraction (logits are O(10); exp(-3.4e38) = 0 exactly on the
    ACT LUT), denominator from a ones column augmented into V inside the same
    AV matmul (129-wide moving, only the relevant kv head's V), normalization
    fused into the PSUM drain as a per-partition scale (times observation mask).
  - Emission interleaves attention(chunk i-1) with projection(chunk i): the PE
    executes its queue in order, so alternating independent streams fills each
    stream's dependency bubbles and keeps the HAM clock gate warm.
"""

from contextlib import ExitStack

import ml_dtypes
import numpy as np

import concourse.tile as tile
from concourse import bacc, mybir
from concourse.bass_utils import run_bass_kernel_spmd

F32 = mybir.dt.float32
BF16 = mybir.dt.bfloat16

B, N, C = 2, 2048, 4096
NUM_HEADS, KV_HEADS, HEAD_DIM = 32, 8, 128
GROUPS = 4                     # head groups per batch element
QH = NUM_HEADS // GROUPS       # 8 q heads per core
KVH = KV_HEADS // GROUPS       # 2 kv heads per core
N_CORES = B * GROUPS

NCHUNK = 512                   # seq positions per pass
NCHUNKS = N // NCHUNK          # 4
CC = C // 128                  # 32 contraction chunks
NT = N // 128                  # 16 position tiles
VW = 130                       # v block width in vaug: [v(128) | one | pad]


def _build_program():
    nc = bacc.Bacc("TRN2", target_bir_lowering=False, debug=False, num_devices=N_CORES)

    xt = nc.dram_tensor("xt", [NCHUNKS, 8, 128, 4, NCHUNK], BF16, kind="ExternalInput").ap()
    wt = nc.dram_tensor("wt", [6, 128, 8, 4, 256], BF16, kind="ExternalInput").ap()
    cosT = nc.dram_tensor("cosT", [128, N], F32, kind="ExternalInput").ap()
    sinmT = nc.dram_tensor("sinmT", [128, N], F32, kind="ExternalInput").ap()
    maskd = nc.dram_tensor("maskd", [NCHUNKS, 128, 4, 128], F32, kind="ExternalInput").ap()
    obs = nc.dram_tensor("obs", [128, NT], F32, kind="ExternalInput").ap()
    rmat = nc.dram_tensor("rmat", [128, 128], BF16, kind="ExternalInput").ap()
    out = nc.dram_tensor("out", [N, QH * 128], BF16, kind="ExternalOutput").ap()

    with tile.TileContext(nc) as tc, ExitStack() as ctx:
        singles = ctx.enter_context(tc.tile_pool(name="singles", bufs=1))
        xpool = ctx.enter_context(tc.tile_pool(name="xt", bufs=9))
        rpool = ctx.enter_context(tc.tile_pool(name="rope", bufs=2))
        cspool = ctx.enter_context(tc.tile_pool(name="cs", bufs=2))
        qtpool = ctx.enter_context(tc.tile_pool(name="qt", bufs=2))
        ptpool = ctx.enter_context(tc.tile_pool(name="pt", bufs=5))
        mpool = ctx.enter_context(tc.tile_pool(name="mask", bufs=1))
        opool = ctx.enter_context(tc.tile_pool(name="outp", bufs=4))
        spool = ctx.enter_context(tc.tile_pool(name="small", bufs=8))
        pp_proj = ctx.enter_context(tc.tile_pool(name="pp_proj", bufs=2, space="PSUM"))
        pp_misc = ctx.enter_context(tc.tile_pool(name="pp_misc", bufs=2, space="PSUM"))
        pp_av = ctx.enter_context(tc.tile_pool(name="pp_av", bufs=4, space="PSUM"))

        # ---- persistent tiles (DMAs emitted after chunk 0 xt kick off) ----
        rmat_sb = singles.tile([128, 128], BF16, tag="rmat")
        obs_sb = singles.tile([128, NT], F32, tag="obs")
        ones_sb = singles.tile([128, NT, 2], BF16, tag="ones")
        kt_res = singles.tile([128, KVH, N], BF16, tag="ktres")      # K^T rope'd
        vaug = singles.tile([128, NT, 2 * VW], BF16, tag="vaug")     # V + ones cols
        wsb = [singles.tile([128, 8, 4, 256], BF16, tag=f"wt{fg}", name=f"wsb{fg}")
               for fg in range(6)]

        def emit_singles_early():
            # quarter-granularity fg0 DMAs so the first matmuls start early
            for q in range(4):
                nc.sync.dma_start(out=wsb[0][:, 2 * q:2 * q + 2],
                                  in_=wt[0][:, 2 * q:2 * q + 2])
            nc.sync.dma_start(out=rmat_sb, in_=rmat)

        def emit_singles_late():
            # fg1 now; fg2/fg3 deferred into the chunk-0 unit stream so the
            # sync queue never backlogs ahead of later small transfers
            nc.sync.dma_start(out=wsb[1], in_=wt[1])
            for fg in (5, 4):
                nc.gpsimd.dma_start(out=wsb[fg], in_=wt[fg])
            nc.gpsimd.dma_start(out=obs_sb, in_=obs)
            nc.vector.memset(ones_sb, 1.0)
            nc.scalar.copy(vaug[:, :, 128:130], ones_sb)
            nc.scalar.copy(vaug[:, :, VW + 128:VW + 130], ones_sb)

        def rope_pair(ps_a, ps_b, dest_a, dest_b, cos_c, sinm_c):
            """dest = ps*cos + (R @ ps)*sinm for two head chunks."""
            qq_a = rpool.tile([128, NCHUNK], BF16, tag="qq")
            nc.vector.tensor_scalar_mul(qq_a, ps_a, 1.0)
            qq_b = rpool.tile([128, NCHUNK], BF16, tag="qq2")
            nc.vector.tensor_scalar_mul(qq_b, ps_b, 1.0)
            pr_a = pp_misc.tile([128, NCHUNK], F32, tag="misc")
            nc.tensor.matmul(pr_a, rmat_sb, qq_a, start=True, stop=True)
            pr_b = pp_misc.tile([128, NCHUNK], F32, tag="misc")
            nc.tensor.matmul(pr_b, rmat_sb, qq_b, start=True, stop=True)
            for ps, pr, qq, dest, tg in ((ps_a, pr_a, qq_a, dest_a, "a"),
                                         (ps_b, pr_b, qq_b, dest_b, "b")):
                tcs = rpool.tile([128, NCHUNK], F32, tag="tcs" + tg, bufs=1)
                nc.vector.tensor_mul(tcs, ps, cos_c)
                # qq is dead after the R-matmul; reuse its slot for the sin product
                nc.vector.tensor_mul(qq, pr, sinm_c)
                nc.vector.tensor_add(dest, tcs, qq)

        def chunk_dmas(qc):
            """Emit the input DMAs for chunk qc; returns chunk state."""
            n0 = qc * NCHUNK
            engs = [nc.gpsimd, nc.scalar]
            xsub = []
            for j in range(8):
                t = xpool.tile([128, 4, NCHUNK], BF16, tag="xt", name=f"xt{qc}_{j}")
                # spread the tile fetches across parallel DMA queues
                engs[j % len(engs)].dma_start(out=t, in_=xt[qc, j])
                xsub.append(t)
            cos_c = cspool.tile([128, NCHUNK], F32, tag="cos", name=f"cos{qc}")
            nc.gpsimd.dma_start(out=cos_c, in_=cosT[:, n0:n0 + NCHUNK])
            sinm_c = cspool.tile([128, NCHUNK], F32, tag="sinm", name=f"sinm{qc}")
            nc.gpsimd.dma_start(out=sinm_c, in_=sinmT[:, n0:n0 + NCHUNK])
            qt_chunk = qtpool.tile([128, QH, NCHUNK], BF16, tag="qt", name=f"qt{qc}")
            return dict(n0=n0, xsub=xsub, cos=cos_c, sinm=sinm_c, qt=qt_chunk)

        def proj_units(qc, st):
            """7 emission units: 5 feature-group c-loops (+deferred rope), 2 V passes."""
            n0, xsub = st["n0"], st["xsub"]

            def xc(c):
                return xsub[c // 4][:, c % 4, :]

            pend = {}

            def emit_fg(fg, half):
                if half == 0:
                    pend[fg] = (
                        pp_proj.tile([128, NCHUNK], F32, tag="proj", name=f"pa{qc}_{fg}"),
                        pp_proj.tile([128, NCHUNK], F32, tag="proj", name=f"pb{qc}_{fg}"))
                ps_a, ps_b = pend[fg]
                for cq in range(4 * half, 4 * half + 4):
                    for i in range(4):
                        c = 4 * cq + i
                        nc.tensor.matmul(ps_a, wsb[fg][:, cq, i, 0:128], xc(c),
                                         start=(c == 0), stop=(c == CC - 1))
                        nc.tensor.matmul(ps_b, wsb[fg][:, cq, i, 128:256], xc(c),
                                         start=(c == 0), stop=(c == CC - 1))

            def emit_rope(fg):
                ps_a, ps_b = pend.pop(fg)
                if fg < 4:
                    d_a = st["qt"][:, 2 * fg, :]
                    d_b = st["qt"][:, 2 * fg + 1, :]
                else:
                    d_a = kt_res[:, 0, n0:n0 + NCHUNK]
                    d_b = kt_res[:, 1, n0:n0 + NCHUNK]
                rope_pair(ps_a, ps_b, d_a, d_b, st["cos"], st["sinm"])

            vps = {}

            def emit_vhalf(p, half):
                if half == 0:
                    vps[p] = [pp_proj.tile([128, 256], F32, tag="proj",
                                           name=f"pv{qc}_{p}_{i}") for i in range(2)]
                ps = vps[p]
                for cq in range(4 * half, 4 * half + 4):
                    for i in range(4):
                        c = 4 * cq + i
                        for k in range(2):
                            ns = 2 * p + k
                            nc.tensor.matmul(ps[k], xc(c)[:, 128 * ns:128 * ns + 128],
                                             wsb[5][:, cq, i, :],
                                             start=(c == 0), stop=(c == CC - 1))
                if half == 1:
                    for k in range(2):
                        ns = 2 * p + k
                        nt_i = 4 * qc + ns
                        dst = vaug[:, nt_i, :].rearrange("p (h w) -> p h w", h=2)[:, :, 0:128]
                        nc.scalar.copy(dst, ps[k].rearrange("p (h w) -> p h w", h=2))

            units = [lambda: emit_fg(0, 0), lambda: emit_fg(0, 1)]
            for fg in range(1, 5):
                units.append(lambda fg=fg: (emit_rope(fg - 1), emit_fg(fg, 0)))
                units.append(lambda fg=fg: emit_fg(fg, 1))
            units.append(lambda: (emit_rope(4), emit_vhalf(0, 0)))
            units.append(lambda: emit_vhalf(0, 1))
            units.append(lambda: emit_vhalf(1, 0))
            units.append(lambda: emit_vhalf(1, 1))
            if qc == 0:
                # deferred weight fetches: emitted two units before first use,
                # on whichever queue drains soonest
                for idx, fg, eng in ((2, 2, nc.scalar), (4, 3, nc.sync)):
                    prev = units[idx]
                    units[idx] = (lambda prev=prev, fg=fg, eng=eng:
                                  (eng.dma_start(out=wsb[fg], in_=wt[fg]), prev()))
            return units

        def attn_units(qc, st):
            """9 emission units: mask/out setup + 8 heads; last head DMAs out."""
            n0, qt_chunk = st["n0"], st["qt"]
            nk = 4 * qc + 4
            shared = {}

            def emit_pre():
                m_sb = mpool.tile([128, 4, 128], F32, tag="mask", name=f"m{qc}")
                nc.gpsimd.dma_start(out=m_sb, in_=maskd[qc])
                shared["m"] = m_sb
                shared["outs"] = [opool.tile([128, QH * 128], BF16, tag="out",
                                             name=f"out_s{qc}_{s}") for s in range(4)]

            def emit_head(h):
                kv = h // (QH // KVH)
                vbase = kv * VW
                m_sb, out_s = shared["m"], shared["outs"]
                pt_tiles = {}
                # last chunk has no projection interleaved: borrow the idle
                # proj psum pool too, so 4 S tiles can be in flight
                last = qc == NCHUNKS - 1
                STAG = 3 if last else 2

                def qk_step(kt):
                    # columns left of q-sub kd are never consumed (per-s AV depth)
                    kd = kt - 4 * qc
                    lo = 128 * kd if kd > 0 else 0
                    spsum = pp_proj if (last and kt % 2) else pp_misc
                    ps_s = spsum.tile([128, NCHUNK], F32, tag="misc" if spsum is pp_misc
                                      else "proj", name=f"s{qc}_{h}_{kt}")
                    nc.tensor.matmul(ps_s[:, lo:], kt_res[:, kv, 128 * kt:128 * kt + 128],
                                     qt_chunk[:, h, lo:], start=True, stop=True)
                    if kd >= 0:
                        # causal mask inside the diagonal 128x128 block only
                        nc.vector.tensor_add(ps_s[:, 128 * kd:128 * kd + 128],
                                             ps_s[:, 128 * kd:128 * kd + 128],
                                             m_sb[:, kd, :])
                    pt = ptpool.tile([128, NCHUNK], BF16, tag="pt", name=f"pt{qc}_{h}_{kt}")
                    nc.scalar.activation(out=pt[:, lo:], in_=ps_s[:, lo:],
                                         func=mybir.ActivationFunctionType.Exp)
                    pt_tiles[kt] = pt

                # one accumulation group per PSUM bank: start_tensor_calc
                # resets the whole bank, so groups cannot share one
                ps_o = [pp_av.tile([128, VW - 1], F32, tag="av", name=f"o{qc}_{h}_{s}")
                        for s in range(4)]

                def av_step(kt):
                    pt = pt_tiles.pop(kt)
                    for s in range(4):
                        if kt > 4 * qc + s:
                            continue  # beyond this q-sub's causal depth
                        nc.tensor.matmul(ps_o[s],
                                         pt[:, 128 * s:128 * s + 128],
                                         vaug[:, kt, vbase:vbase + VW - 1],
                                         start=(kt == 4 * qc),
                                         stop=(kt == (4 * qc - 1 if qc else s)))

                # diagonal tiles first: their DVE mask hop pipelines while the
                # stagger is filling, leaving the steady state mask-free
                kts = list(range(4 * qc, nk)) + list(range(4 * qc))
                for idx in range(nk + STAG):
                    if idx < nk:
                        qk_step(kts[idx])
                    if idx >= STAG:
                        av_step(kts[idx - STAG])

                # drain the AV banks with one DVE copy each so the next head's
                # accumulation can start; normalize from SBUF off the PE and
                # ACT paths (gpsimd cannot read PSUM)
                avsb = opool.tile([128, 4, VW - 1], F32, tag="avsb",
                                  name=f"avsb{qc}_{h}", bufs=2)
                for s in range(4):
                    nc.vector.tensor_scalar_mul(avsb[:, s, :], ps_o[s], 1.0)
                for s in range(4):
                    den = spool.tile([128, 1], F32, tag="den")
                    nc.vector.reciprocal(den, avsb[:, s, 128:129])
                    sc = spool.tile([128, 1], F32, tag="sc")
                    nc.vector.tensor_mul(sc, den, obs_sb[:, 4 * qc + s:4 * qc + s + 1])
                    nc.vector.tensor_scalar_mul(out_s[s][:, 128 * h:128 * h + 128],
                                                avsb[:, s, 0:128], sc)
                if h == QH // 2 - 1:
                    # first half of the heads is done: drain those columns early
                    for s in range(4):
                        nc.scalar.dma_start(
                            out=out[n0 + 128 * s:n0 + 128 * (s + 1), 0:QH * 64],
                            in_=out_s[s][:, 0:QH * 64])
                if h == QH - 1:
                    for s in range(4):
                        nc.scalar.dma_start(
                            out=out[n0 + 128 * s:n0 + 128 * (s + 1), QH * 64:],
                            in_=out_s[s][:, QH * 64:])

            units = [emit_pre]
            for h in range(QH):
                units.append(lambda h=h: emit_head(h))
            return units

        # ---- pipelined emission: attention(qc-1) interleaved with proj(qc) ----
        emit_singles_early()
        states = {0: chunk_dmas(0)}
        emit_singles_late()
        for u in proj_units(0, states[0]):
            u()
        for qc in range(1, NCHUNKS + 1):
            au = attn_units(qc - 1, states[qc - 1])
            pu = []
            if qc < NCHUNKS:
                states[qc] = chunk_dmas(qc)
                pu = proj_units(qc, states[qc])
            # proportional merge of the two unit streams
            tagged = [((i + 0.5) / len(au), 0, u) for i, u in enumerate(au)]
            tagged += [((j + 0.5) / len(pu), 1, u) for j, u in enumerate(pu)]
            for _, _, u in sorted(tagged, key=lambda t: (t[0], t[1])):
                u()
            del states[qc - 1]

    nc.compile()
    return nc


_NC_CACHE = None


def _get_program():
    global _NC_CACHE
    if _NC_CACHE is None:
        _NC_CACHE = _build_program()
    return _NC_CACHE


def _prep_core_inputs(input, weight, cos_cached, sin_cached, attention_mask,
                      position_ids, observation_mask):
    """Build the 8 per-core input maps (host-side shard + layout + bf16 cast)."""
    input = np.asarray(input, dtype=np.float32)
    weight = np.asarray(weight, dtype=np.float32)
    cos_cached = np.asarray(cos_cached, dtype=np.float32)
    sin_cached = np.asarray(sin_cached, dtype=np.float32)
    attention_mask = np.asarray(attention_mask, dtype=np.float32)
    position_ids = np.asarray(position_ids)
    observation_mask = np.asarray(observation_mask)

    scale = 1.0 / np.sqrt(HEAD_DIM)
    rmat = np.zeros((128, 128), dtype=np.float32)
    idx = np.arange(128)
    rmat[idx, (idx + 64) % 128] = 1.0
    rmat = rmat.astype(ml_dtypes.bfloat16)

    in_maps = []
    for core in range(N_CORES):
        b, g = core // GROUPS, core % GROUPS
        xtT = input[b].T.astype(ml_dtypes.bfloat16)                    # [C, N]
        xt = np.ascontiguousarray(
            xtT.reshape(8, 4, 128, NCHUNKS, NCHUNK).transpose(3, 0, 2, 1, 4))

        wq = weight[g * QH * 128:(g + 1) * QH * 128] * scale           # [1024, C]
        k_off = NUM_HEADS * 128
        wk = weight[k_off + g * KVH * 128:k_off + (g + 1) * KVH * 128]  # [256, C]
        v_off = k_off + KV_HEADS * 128
        wv = weight[v_off + g * KVH * 128:v_off + (g + 1) * KVH * 128]  # [256, C]
        wtT = np.concatenate([wq, wk, wv], axis=0).T.astype(ml_dtypes.bfloat16)
        wt = np.ascontiguousarray(
            wtT.reshape(8, 4, 128, 6, 256).transpose(3, 2, 0, 1, 4))   # [6,128,8,4,256]

        pos = position_ids[b]
        cosT = np.ascontiguousarray(cos_cached[0, 0][pos].T)           # [128, N]
        sinmT = np.ascontiguousarray(sin_cached[0, 0][pos].T)
        sinmT[:64] = -sinmT[:64]

        m = attention_mask[b, 0]                                       # [N, N]
        maskd = np.stack([
            np.stack([m[t * 128:(t + 1) * 128, t * 128:(t + 1) * 128].T
                      for t in range(4 * qc, 4 * qc + 4)]).transpose(1, 0, 2)
            for qc in range(NCHUNKS)])                                 # [4, 128, 4, 128]
        maskd = np.ascontiguousarray(maskd)

        obsf = np.ascontiguousarray(
            (observation_mask[b] == 1).astype(np.float32).reshape(NT, 128).T)

        in_maps.append(dict(xt=xt, wt=wt, cosT=cosT, sinmT=sinmT, maskd=maskd,
                            obs=obsf, rmat=rmat))
    return in_maps


def run(inputs: dict, trace: bool = False):
    """Run the sharded kernel; returns (full_output [B*N, C] fp32, BassKernelResults)."""
    nc = _get_program()
    in_maps = _prep_core_inputs(**inputs)
    res = run_bass_kernel_spmd(nc, in_maps, core_ids=list(range(N_CORES)), trace=trace)
    full = np.empty((B, N, C), dtype=np.float32)
    for core in range(N_CORES):
        b, g = core // GROUPS, core % GROUPS
        full[b, :, g * QH * 128:(g + 1) * QH * 128] = res.results[core]["out"].astype(
            np.float32)
    return full.reshape(B * N, C), res


def kernel(**inputs) -> np.ndarray:
    out, _ = run(inputs)
    return out


# revision 30
# speedup vs baseline: 1.1994x; 1.0366x over previous
"""Fused QKV+RoPE+GQA-attention kernel for Trainium2, sharded over 8 NeuronCores.

Sharding: data-parallel over batch (B=2), tensor-parallel over heads (4 groups of
8 q-heads / 2 kv-heads per batch element).  Each core computes its output slice
out[b, :, g*1024:(g+1)*1024] independently; no collectives.

Per-core pipeline (n=2048 seq, c=4096 model, d=128 head dim), all matmul
operands in bf16 (full PE rate at any moving width, half the DMA/SBUF of
fp32r):
  - Weights live SBUF-resident for the whole kernel (12.6 MB bf16, DMA'd once
    up front) instead of being re-streamed per position chunk.
  - QKV^T projection:  Q^T/K^T = W X^T (feature-major, head_dim on partitions),
    V = X W^T (position-major).
  - RoPE without partition shifts: rot_half(q) = sign * (R q) where R is the
    swap-halves permutation applied via one extra matmul per head-chunk, and the
    sign is folded into the sin table on host.
  - Attention: S^T = K_rope Q_rope^T (k on partitions, q on free dim), causal
    structure exploited at 128-tile granularity (per-q-subtile AV depth), the
    real attention-mask applied only inside diagonal 128x128 blocks, softmax
    without max-subtraction (logits are O(10); exp(-3.4e38) = 0 exactly on the
    ACT LUT), denominator from a ones column augmented into V inside the same
    AV matmul (129-wide moving, only the relevant kv head's V), normalization
    fused into the PSUM drain as a per-partition scale (times observation mask).
  - Emission interleaves attention(chunk i-1) with projection(chunk i): the PE
    executes its queue in order, so alternating independent streams fills each
    stream's dependency bubbles and keeps the HAM clock gate warm.
"""

from contextlib import ExitStack

import ml_dtypes
import numpy as np

import concourse.tile as tile
from concourse import bacc, mybir
from concourse.bass_utils import run_bass_kernel_spmd

F32 = mybir.dt.float32
BF16 = mybir.dt.bfloat16

B, N, C = 2, 2048, 4096
NUM_HEADS, KV_HEADS, HEAD_DIM = 32, 8, 128
GROUPS = 4                     # head groups per batch element
QH = NUM_HEADS // GROUPS       # 8 q heads per core
KVH = KV_HEADS // GROUPS       # 2 kv heads per core
N_CORES = B * GROUPS

NCHUNK = 512                   # seq positions per pass
NCHUNKS = N // NCHUNK          # 4
CC = C // 128                  # 32 contraction chunks
NT = N // 128                  # 16 position tiles
VW = 130                       # v block width in vaug: [v(128) | one | pad]


def _build_program():
    nc = bacc.Bacc("TRN2", target_bir_lowering=False, debug=False, num_devices=N_CORES)

    xt = nc.dram_tensor("xt", [NCHUNKS, 8, 128, 4, NCHUNK], BF16, kind="ExternalInput").ap()
    wt = nc.dram_tensor("wt", [6, 128, 8, 4, 256], BF16, kind="ExternalInput").ap()
    cosT = nc.dram_tensor("cosT", [128, N], F32, kind="ExternalInput").ap()
    sinmT = nc.dram_tensor("sinmT", [128, N], F32, kind="ExternalInput").ap()
    maskd = nc.dram_tensor("maskd", [NCHUNKS, 128, 4, 128], F32, kind="ExternalInput").ap()
    obs = nc.dram_tensor("obs", [128, NT], F32, kind="ExternalInput").ap()
    rmat = nc.dram_tensor("rmat", [128, 128], BF16, kind="ExternalInput").ap()
    out = nc.dram_tensor("out", [N, QH * 128], BF16, kind="ExternalOutput").ap()

    with tile.TileContext(nc) as tc, ExitStack() as ctx:
        singles = ctx.enter_context(tc.tile_pool(name="singles", bufs=1))
        xpool = ctx.enter_context(tc.tile_pool(name="xt", bufs=9))
        rpool = ctx.enter_context(tc.tile_pool(name="rope", bufs=2))
        cspool = ctx.enter_context(tc.tile_pool(name="cs", bufs=2))
        qtpool = ctx.enter_context(tc.tile_pool(name="qt", bufs=2))
        ptpool = ctx.enter_context(tc.tile_pool(name="pt", bufs=6))
        mpool = ctx.enter_context(tc.tile_pool(name="mask", bufs=1))
        opool = ctx.enter_context(tc.tile_pool(name="outp", bufs=4))
        spool = ctx.enter_context(tc.tile_pool(name="small", bufs=8))
        pp_proj = ctx.enter_context(tc.tile_pool(name="pp_proj", bufs=2, space="PSUM"))
        pp_misc = ctx.enter_context(tc.tile_pool(name="pp_misc", bufs=2, space="PSUM"))
        pp_av = ctx.enter_context(tc.tile_pool(name="pp_av", bufs=4, space="PSUM"))

        # ---- persistent tiles (DMAs emitted after chunk 0 xt kick off) ----
        rmat_sb = singles.tile([128, 128], BF16, tag="rmat")
        obs_sb = singles.tile([128, NT], F32, tag="obs")
        ones_sb = singles.tile([128, NT, 2], BF16, tag="ones")
        kt_res = singles.tile([128, KVH, N], BF16, tag="ktres")      # K^T rope'd
        vaug = singles.tile([128, NT, 2 * VW], BF16, tag="vaug")     # V + ones cols
        wsb = [singles.tile([128, 8, 4, 256], BF16, tag=f"wt{fg}", name=f"wsb{fg}")
               for fg in range(6)]

        def emit_coldstart(st):
            """Chunk-0 inputs, round-robin across the 3 DMA queues in strict
            first-use order: per-queue bandwidth (~170GB/s) is the cold-start
            limiter, so every queue must deliver exactly what the PE needs
            next.  Weights fg0-fg3 go at quarter granularity."""
            xfers = []

            def wq(fg, q):
                xfers.append((wsb[fg][:, 2 * q:2 * q + 2],
                              wt[fg][:, 2 * q:2 * q + 2]))

            wq(0, 0)
            xfers.append((st["xsub"][0], xt[0, 0]))
            xfers.append((st["xsub"][1], xt[0, 1]))
            wq(0, 1)
            xfers.append((st["xsub"][2], xt[0, 2]))
            xfers.append((st["xsub"][3], xt[0, 3]))
            wq(0, 2)
            xfers.append((st["xsub"][4], xt[0, 4]))
            xfers.append((st["xsub"][5], xt[0, 5]))
            wq(0, 3)
            xfers.append((st["xsub"][6], xt[0, 6]))
            xfers.append((st["xsub"][7], xt[0, 7]))
            xfers.append((rmat_sb, rmat))
            for q in range(4):
                wq(1, q)
            xfers.append((st["cos"], cosT[:, 0:NCHUNK]))
            xfers.append((st["sinm"], sinmT[:, 0:NCHUNK]))
            for q in range(4):
                wq(2, q)
            for q in range(4):
                wq(3, q)
            xfers.append((wsb[5], wt[5]))
            xfers.append((wsb[4], wt[4]))
            xfers.append((obs_sb, obs))
            engs = [nc.sync, nc.gpsimd, nc.scalar]
            for i, (dst, src) in enumerate(xfers):
                engs[i % 3].dma_start(out=dst, in_=src)
            nc.vector.memset(ones_sb, 1.0)
            nc.scalar.copy(vaug[:, :, 128:130], ones_sb)
            nc.scalar.copy(vaug[:, :, VW + 128:VW + 130], ones_sb)

        def rope_pair(ps_a, ps_b, dest_a, dest_b, cos_c, sinm_c):
            """dest = ps*cos + (R @ ps)*sinm for two head chunks."""
            qq_a = rpool.tile([128, NCHUNK], BF16, tag="qq")
            nc.vector.tensor_scalar_mul(qq_a, ps_a, 1.0)
            qq_b = rpool.tile([128, NCHUNK], BF16, tag="qq2")
            nc.vector.tensor_scalar_mul(qq_b, ps_b, 1.0)
            pr_a = pp_misc.tile([128, NCHUNK], F32, tag="misc")
            nc.tensor.matmul(pr_a, rmat_sb, qq_a, start=True, stop=True)
            pr_b = pp_misc.tile([128, NCHUNK], F32, tag="misc")
            nc.tensor.matmul(pr_b, rmat_sb, qq_b, start=True, stop=True)
            for ps, pr, qq, dest, tg in ((ps_a, pr_a, qq_a, dest_a, "a"),
                                         (ps_b, pr_b, qq_b, dest_b, "b")):
                tcs = rpool.tile([128, NCHUNK], F32, tag="tcs" + tg, bufs=1)
                nc.vector.tensor_mul(tcs, ps, cos_c)
                # qq is dead after the R-matmul; reuse its slot for the sin product
                nc.vector.tensor_mul(qq, pr, sinm_c)
                nc.vector.tensor_add(dest, tcs, qq)

        def chunk_dmas(qc):
            """Emit the input DMAs for chunk qc; returns chunk state.
            Chunk 0 only allocates tiles — emit_coldstart orders its DMAs."""
            n0 = qc * NCHUNK
            engs = [nc.gpsimd, nc.scalar]
            xsub = []
            for j in range(8):
                t = xpool.tile([128, 4, NCHUNK], BF16, tag="xt", name=f"xt{qc}_{j}")
                if qc > 0:
                    # spread the tile fetches across parallel DMA queues
                    engs[j % len(engs)].dma_start(out=t, in_=xt[qc, j])
                xsub.append(t)
            cos_c = cspool.tile([128, NCHUNK], F32, tag="cos", name=f"cos{qc}")
            sinm_c = cspool.tile([128, NCHUNK], F32, tag="sinm", name=f"sinm{qc}")
            if qc > 0:
                nc.gpsimd.dma_start(out=cos_c, in_=cosT[:, n0:n0 + NCHUNK])
                nc.gpsimd.dma_start(out=sinm_c, in_=sinmT[:, n0:n0 + NCHUNK])
            qt_chunk = qtpool.tile([128, QH, NCHUNK], BF16, tag="qt", name=f"qt{qc}")
            return dict(n0=n0, xsub=xsub, cos=cos_c, sinm=sinm_c, qt=qt_chunk)

        def proj_units(qc, st):
            """7 emission units: 5 feature-group c-loops (+deferred rope), 2 V passes."""
            n0, xsub = st["n0"], st["xsub"]

            def xc(c):
                return xsub[c // 4][:, c % 4, :]

            pend = {}

            def emit_fg(fg, half):
                if half == 0:
                    pend[fg] = (
                        pp_proj.tile([128, NCHUNK], F32, tag="proj", name=f"pa{qc}_{fg}"),
                        pp_proj.tile([128, NCHUNK], F32, tag="proj", name=f"pb{qc}_{fg}"))
                ps_a, ps_b = pend[fg]
                for cq in range(4 * half, 4 * half + 4):
                    for i in range(4):
                        c = 4 * cq + i
                        nc.tensor.matmul(ps_a, wsb[fg][:, cq, i, 0:128], xc(c),
                                         start=(c == 0), stop=(c == CC - 1))
                        nc.tensor.matmul(ps_b, wsb[fg][:, cq, i, 128:256], xc(c),
                                         start=(c == 0), stop=(c == CC - 1))

            def emit_rope(fg):
                ps_a, ps_b = pend.pop(fg)
                if fg < 4:
                    d_a = st["qt"][:, 2 * fg, :]
                    d_b = st["qt"][:, 2 * fg + 1, :]
                else:
                    d_a = kt_res[:, 0, n0:n0 + NCHUNK]
                    d_b = kt_res[:, 1, n0:n0 + NCHUNK]
                rope_pair(ps_a, ps_b, d_a, d_b, st["cos"], st["sinm"])

            vps = {}

            def emit_vhalf(p, half):
                if half == 0:
                    vps[p] = [pp_proj.tile([128, 256], F32, tag="proj",
                                           name=f"pv{qc}_{p}_{i}") for i in range(2)]
                ps = vps[p]
                for cq in range(4 * half, 4 * half + 4):
                    for i in range(4):
                        c = 4 * cq + i
                        for k in range(2):
                            ns = 2 * p + k
                            nc.tensor.matmul(ps[k], xc(c)[:, 128 * ns:128 * ns + 128],
                                             wsb[5][:, cq, i, :],
                                             start=(c == 0), stop=(c == CC - 1))
                if half == 1:
                    for k in range(2):
                        ns = 2 * p + k
                        nt_i = 4 * qc + ns
                        dst = vaug[:, nt_i, :].rearrange("p (h w) -> p h w", h=2)[:, :, 0:128]
                        nc.scalar.copy(dst, ps[k].rearrange("p (h w) -> p h w", h=2))

            units = [lambda: emit_fg(0, 0), lambda: emit_fg(0, 1)]
            for fg in range(1, 5):
                units.append(lambda fg=fg: (emit_rope(fg - 1), emit_fg(fg, 0)))
                units.append(lambda fg=fg: emit_fg(fg, 1))
            units.append(lambda: (emit_rope(4), emit_vhalf(0, 0)))
            units.append(lambda: emit_vhalf(0, 1))
            units.append(lambda: emit_vhalf(1, 0))
            units.append(lambda: emit_vhalf(1, 1))
            return units

        def attn_units(qc, st):
            """9 emission units: mask/out setup + 8 heads; last head DMAs out."""
            n0, qt_chunk = st["n0"], st["qt"]
            nk = 4 * qc + 4
            shared = {}

            def emit_pre():
                m_sb = mpool.tile([128, 4, 128], F32, tag="mask", name=f"m{qc}")
                nc.gpsimd.dma_start(out=m_sb, in_=maskd[qc])
                shared["m"] = m_sb
                shared["outs"] = [opool.tile([128, QH * 128], BF16, tag="out",
                                             name=f"out_s{qc}_{s}") for s in range(4)]

            def emit_head(h):
                kv = h // (QH // KVH)
                vbase = kv * VW
                m_sb, out_s = shared["m"], shared["outs"]
                pt_tiles = {}
                # last chunk has no projection interleaved: borrow the idle
                # proj psum pool too, so 4 S tiles can be in flight
                last = qc == NCHUNKS - 1
                STAG = 4 if last else 2

                def qk_step(kt):
                    # columns left of q-sub kd are never consumed (per-s AV depth)
                    kd = kt - 4 * qc
                    lo = 128 * kd if kd > 0 else 0
                    spsum = pp_proj if (last and kt % 2) else pp_misc
                    ps_s = spsum.tile([128, NCHUNK], F32, tag="misc" if spsum is pp_misc
                                      else "proj", name=f"s{qc}_{h}_{kt}")
                    nc.tensor.matmul(ps_s[:, lo:], kt_res[:, kv, 128 * kt:128 * kt + 128],
                                     qt_chunk[:, h, lo:], start=True, stop=True)
                    if kd >= 0:
                        # causal mask inside the diagonal 128x128 block only
                        nc.vector.tensor_add(ps_s[:, 128 * kd:128 * kd + 128],
                                             ps_s[:, 128 * kd:128 * kd + 128],
                                             m_sb[:, kd, :])
                    pt = ptpool.tile([128, NCHUNK], BF16, tag="pt", name=f"pt{qc}_{h}_{kt}")
                    nc.scalar.activation(out=pt[:, lo:], in_=ps_s[:, lo:],
                                         func=mybir.ActivationFunctionType.Exp)
                    pt_tiles[kt] = pt

                # one accumulation group per PSUM bank: start_tensor_calc
                # resets the whole bank, so groups cannot share one
                ps_o = [pp_av.tile([128, VW - 1], F32, tag="av", name=f"o{qc}_{h}_{s}")
                        for s in range(4)]

                def av_step(kt):
                    pt = pt_tiles.pop(kt)
                    for s in range(4):
                        if kt > 4 * qc + s:
                            continue  # beyond this q-sub's causal depth
                        nc.tensor.matmul(ps_o[s],
                                         pt[:, 128 * s:128 * s + 128],
                                         vaug[:, kt, vbase:vbase + VW - 1],
                                         start=(kt == 4 * qc),
                                         stop=(kt == (4 * qc - 1 if qc else s)))

                # diagonal tiles first: their DVE mask hop pipelines while the
                # stagger is filling, leaving the steady state mask-free
                kts = list(range(4 * qc, nk)) + list(range(4 * qc))
                for idx in range(nk + STAG):
                    if idx < nk:
                        qk_step(kts[idx])
                    if idx >= STAG:
                        av_step(kts[idx - STAG])

                # drain the AV banks with one DVE copy each so the next head's
                # accumulation can start; normalize from SBUF off the PE and
                # ACT paths (gpsimd cannot read PSUM)
                avsb = opool.tile([128, 4, VW - 1], F32, tag="avsb",
                                  name=f"avsb{qc}_{h}", bufs=2)
                for s in range(4):
                    nc.vector.tensor_scalar_mul(avsb[:, s, :], ps_o[s], 1.0)
                for s in range(4):
                    den = spool.tile([128, 1], F32, tag="den")
                    nc.vector.reciprocal(den, avsb[:, s, 128:129])
                    sc = spool.tile([128, 1], F32, tag="sc")
                    nc.vector.tensor_mul(sc, den, obs_sb[:, 4 * qc + s:4 * qc + s + 1])
                    nc.vector.tensor_scalar_mul(out_s[s][:, 128 * h:128 * h + 128],
                                                avsb[:, s, 0:128], sc)
                if h == QH // 2 - 1:
                    # first half of the heads is done: drain those columns early
                    for s in range(4):
                        nc.scalar.dma_start(
                            out=out[n0 + 128 * s:n0 + 128 * (s + 1), 0:QH * 64],
                            in_=out_s[s][:, 0:QH * 64])
                if h == QH - 1:
                    for s in range(4):
                        nc.scalar.dma_start(
                            out=out[n0 + 128 * s:n0 + 128 * (s + 1), QH * 64:],
                            in_=out_s[s][:, QH * 64:])

            units = [emit_pre]
            for h in range(QH):
                units.append(lambda h=h: emit_head(h))
            return units

        # ---- pipelined emission: attention(qc-1) interleaved with proj(qc) ----
        states = {0: chunk_dmas(0)}
        emit_coldstart(states[0])
        for u in proj_units(0, states[0]):
            u()
        for qc in range(1, NCHUNKS + 1):
            au = attn_units(qc - 1, states[qc - 1])
            pu = []
            if qc < NCHUNKS:
                states[qc] = chunk_dmas(qc)
                pu = proj_units(qc, states[qc])
            # proportional merge of the two unit streams
            tagged = [((i + 0.5) / len(au), 0, u) for i, u in enumerate(au)]
            tagged += [((j + 0.5) / len(pu), 1, u) for j, u in enumerate(pu)]
            for _, _, u in sorted(tagged, key=lambda t: (t[0], t[1])):
                u()
            del states[qc - 1]

    nc.compile()
    return nc


_NC_CACHE = None


def _get_program():
    global _NC_CACHE
    if _NC_CACHE is None:
        _NC_CACHE = _build_program()
    return _NC_CACHE


def _prep_core_inputs(input, weight, cos_cached, sin_cached, attention_mask,
                      position_ids, observation_mask):
    """Build the 8 per-core input maps (host-side shard + layout + bf16 cast)."""
    input = np.asarray(input, dtype=np.float32)
    weight = np.asarray(weight, dtype=np.float32)
    cos_cached = np.asarray(cos_cached, dtype=np.float32)
    sin_cached = np.asarray(sin_cached, dtype=np.float32)
    attention_mask = np.asarray(attention_mask, dtype=np.float32)
    position_ids = np.asarray(position_ids)
    observation_mask = np.asarray(observation_mask)

    scale = 1.0 / np.sqrt(HEAD_DIM)
    rmat = np.zeros((128, 128), dtype=np.float32)
    idx = np.arange(128)
    rmat[idx, (idx + 64) % 128] = 1.0
    rmat = rmat.astype(ml_dtypes.bfloat16)

    in_maps = []
    for core in range(N_CORES):
        b, g = core // GROUPS, core % GROUPS
        xtT = input[b].T.astype(ml_dtypes.bfloat16)                    # [C, N]
        xt = np.ascontiguousarray(
            xtT.reshape(8, 4, 128, NCHUNKS, NCHUNK).transpose(3, 0, 2, 1, 4))

        wq = weight[g * QH * 128:(g + 1) * QH * 128] * scale           # [1024, C]
        k_off = NUM_HEADS * 128
        wk = weight[k_off + g * KVH * 128:k_off + (g + 1) * KVH * 128]  # [256, C]
        v_off = k_off + KV_HEADS * 128
        wv = weight[v_off + g * KVH * 128:v_off + (g + 1) * KVH * 128]  # [256, C]
        wtT = np.concatenate([wq, wk, wv], axis=0).T.astype(ml_dtypes.bfloat16)
        wt = np.ascontiguousarray(
            wtT.reshape(8, 4, 128, 6, 256).transpose(3, 2, 0, 1, 4))   # [6,128,8,4,256]

        pos = position_ids[b]
        cosT = np.ascontiguousarray(cos_cached[0, 0][pos].T)           # [128, N]
        sinmT = np.ascontiguousarray(sin_cached[0, 0][pos].T)
        sinmT[:64] = -sinmT[:64]

        m = attention_mask[b, 0]                                       # [N, N]
        maskd = np.stack([
            np.stack([m[t * 128:(t + 1) * 128, t * 128:(t + 1) * 128].T
                      for t in range(4 * qc, 4 * qc + 4)]).transpose(1, 0, 2)
            for qc in range(NCHUNKS)])                                 # [4, 128, 4, 128]
        maskd = np.ascontiguousarray(maskd)

        obsf = np.ascontiguousarray(
            (observation_mask[b] == 1).astype(np.float32).reshape(NT, 128).T)

        in_maps.append(dict(xt=xt, wt=wt, cosT=cosT, sinmT=sinmT, maskd=maskd,
                            obs=obsf, rmat=rmat))
    return in_maps


def run(inputs: dict, trace: bool = False):
    """Run the sharded kernel; returns (full_output [B*N, C] fp32, BassKernelResults)."""
    nc = _get_program()
    in_maps = _prep_core_inputs(**inputs)
    res = run_bass_kernel_spmd(nc, in_maps, core_ids=list(range(N_CORES)), trace=trace)
    full = np.empty((B, N, C), dtype=np.float32)
    for core in range(N_CORES):
        b, g = core // GROUPS, core % GROUPS
        full[b, :, g * QH * 128:(g + 1) * QH * 128] = res.results[core]["out"].astype(
            np.float32)
    return full.reshape(B * N, C), res


def kernel(**inputs) -> np.ndarray:
    out, _ = run(inputs)
    return out
